# revision 3
# baseline (speedup 1.0000x reference)
"""Trainium2 Bass kernel for nn_L2GESRModule.

Reference computation:
    Fh_conv = Fh @ Wh + bh            (dead: only used via ones_like)
    ESF     = ones_like(Fh_conv)
    ix      = clip(i + 1, 0, H-1);  iy = clip(j + 1, 0, W-1)
    Y       = Fl @ Wl + bl
    out[b,i,j,:] = Y[b, min(i+1,H-1), min(j+1,W-1), :]

So the whole problem is one 1x1-conv GEMM on Fl plus a static (+1,+1)
clamped-shift gather, data-parallel over batch (1 batch element per core).
The Fh/Wh/bh branch contributes nothing to the output and is never loaded.

Per-core pipeline (source rows r = 1..H-1, one image row = 128 pixels):
  DMA   : load a chunk of rows of Fl[b] -> SBUF [128px, R, 256cin]
  PE    : 2x transpose (128x128 fp32) -> PSUM  [cin, px]
  ACT   : evacuate PSUM -> SBUF X^T tile
  PE    : 2x matmul (K=128 each) accumulate -> PSUM [px, 256cout]
  DVE   : PSUM + bias -> SBUF
  DMA   : shifted store: SBUF partitions 1..127 -> out[b, r-1, 0:127, :],
          partition 127 -> out[b, r-1, 127, :]; row H-1 stored twice.
"""

import numpy as np

import concourse.bacc as bacc
import concourse.mybir as mybir
from concourse import bass_utils, tile
from concourse.masks import make_identity

B, H, W, CIN, COUT = 8, 128, 128, 256, 256
N_CORES = 8
CHUNK = 16

# mybir.dt.float32r runs the PE at 4x the fp32 rate (1 cycle/row when the
# moving dim is >= 256); fp32 is exact but quarter-rate.
MM_DT = mybir.dt.float32r


def build_nc(n_rows: int = H, mm_dt=MM_DT, chunk: int = CHUNK):
    f32 = mybir.dt.float32
    nc = bacc.Bacc("TRN2", target_bir_lowering=False, debug=False)
    Fl = nc.dram_tensor("Fl", [n_rows, W, CIN], f32, kind="ExternalInput").ap()
    Wl = nc.dram_tensor("Wl", [CIN, COUT], f32, kind="ExternalInput").ap()
    bl = nc.dram_tensor("bl", [COUT], f32, kind="ExternalInput").ap()
    out = nc.dram_tensor("out", [n_rows, W, COUT], f32, kind="ExternalOutput").ap()

    with tile.TileContext(nc) as tc:
        with (
            tc.tile_pool(name="consts", bufs=1) as consts,
            tc.tile_pool(name="xin", bufs=3) as xin_pool,
            tc.tile_pool(name="xt", bufs=3) as xt_pool,
            tc.tile_pool(name="yout", bufs=2) as yout_pool,
            tc.tile_pool(name="pt", bufs=3, space="PSUM") as pt_pool,
            tc.tile_pool(name="py", bufs=3, space="PSUM") as py_pool,
            tc.tile_pool(name="pb", bufs=1, space="PSUM") as pb_pool,
        ):
            ident = consts.tile([128, 128], f32)
            make_identity(nc, ident)

            # Wl as two K-chunks: w_sb[c, kc, n] = Wl[kc*128 + c, n]
            # fp32r matmul operands must be *rounded* to fp32r by their
            # producer (BIR verifier rule), so cast during the DMA (SWDGE).
            w_sb = consts.tile([128, 2, COUT], mm_dt)
            w_src = Wl.rearrange("(kc kp) n -> kp kc n", kp=128)
            if mm_dt == f32:
                nc.sync.dma_start(w_sb, w_src)
            else:
                nc.gpsimd.dma_start(w_sb, w_src)

            # bias broadcast to all 128 partitions via ones[128,1] @ bl[1,256]
            ones = consts.tile([1, 128], f32)
            nc.gpsimd.memset(ones, 1.0)
            bl_sb = consts.tile([1, COUT], f32)
            nc.sync.dma_start(bl_sb, bl[None, :])
            bias_ps = pb_pool.tile([128, COUT], f32)
            nc.tensor.matmul(bias_ps, ones, bl_sb, start=True, stop=True)
            bias_sb = consts.tile([128, COUT], f32)
            nc.scalar.copy(bias_sb, bias_ps)

            rows = list(range(1, n_rows))  # source rows
            for c0 in range(0, len(rows), chunk):
                rchunk = rows[c0 : c0 + chunk]
                R = len(rchunk)
                r0 = rchunk[0]

                xbig = xin_pool.tile([128, R, CIN], f32, tag="xin")
                nc.sync.dma_start(xbig, Fl[r0 : r0 + R].rearrange("r p c -> p r c"))
                ybig = yout_pool.tile([128, R, COUT], f32, tag="yout")

                for j in range(R):
                    pt = pt_pool.tile([128, 2, 128], f32, tag="pt")
                    nc.tensor.transpose(pt[:, 0], xbig[:, j, 0:128], ident)
                    nc.tensor.transpose(pt[:, 1], xbig[:, j, 128:256], ident)
                    xt = xt_pool.tile([128, 2, 128], mm_dt, tag="xt")
                    nc.scalar.copy(xt, pt)

                    py = py_pool.tile([128, COUT], f32, tag="py")
                    nc.tensor.matmul(py, xt[:, 0], w_sb[:, 0], start=True, stop=False)
                    nc.tensor.matmul(py, xt[:, 1], w_sb[:, 1], start=False, stop=True)

                    nc.vector.tensor_add(ybig[:, j], py, bias_sb)

                # column-shifted store: out[r-1, jj] = Y[r, jj+1] for jj<W-1,
                # out[r-1, W-1] = Y[r, W-1]
                nc.sync.dma_start(
                    out[r0 - 1 : r0 - 1 + R, 0 : W - 1, :].rearrange("r p c -> p r c"),
                    ybig[1:128],
                )
                nc.sync.dma_start(out[r0 - 1 : r0 - 1 + R, W - 1, :][None], ybig[127:128])

                if rchunk[-1] == n_rows - 1:
                    # out row H-1 duplicates out row H-2 (both read source row H-1)
                    nc.sync.dma_start(out[n_rows - 1, 0 : W - 1, :], ybig[1:128, R - 1, :])
                    nc.sync.dma_start(out[n_rows - 1, W - 1, :][None], ybig[127:128, R - 1, :])

    nc.compile()
    return nc


_cache: dict = {}


def _get_nc():
    if "nc" not in _cache:
        _cache["nc"] = build_nc()
    return _cache["nc"]


def kernel(Fh, Fl, Wh, bh, Wl, bl):
    nc = _get_nc()
    Fl = np.asarray(Fl, dtype=np.float32)
    Wl_np = np.ascontiguousarray(np.asarray(Wl, dtype=np.float32))
    bl_np = np.ascontiguousarray(np.asarray(bl, dtype=np.float32))
    in_maps = [
        {"Fl": np.ascontiguousarray(Fl[b]), "Wl": Wl_np, "bl": bl_np}
        for b in range(B)
    ]
    res = bass_utils.run_bass_kernel_spmd(nc, in_maps, core_ids=list(range(N_CORES)))
    return np.stack([res.results[b]["out"] for b in range(B)], axis=0)
